# revision 1
# baseline (speedup 1.0000x reference)
"""Trainium2 Bass kernel for the BKT (multi-HMM knowledge tracing) forward model.

Strategy: data-parallel over students (1024 students / 8 cores = 128 per core,
one SBUF partition per student). The T=500 time recursion runs locally per core.

Per-core algebra per step t (all tables precomputed on host):
    c      = A[kc[:,t]]                          [128,100]  (gathered, rows sum to 1)
    G4     = (A @ log_t)[kc[:,t]]                [128,4]    (s,s')  -- in same gather row
    L4     = log_obs[problem[:,t]] in (o,s)      [128,4]
    OLL    = log_obs[problem[:,t]][:, corr]      [128,2]    (s')    -- corr folded into idx
    a2_s   = sum_k c * alpha_s                   (fused tensor_tensor_reduce)
    t010   = [G4 + OLL(s'), L4, 0, 0] + a2-dup   [128,10]
    e10    = exp(t010); ps5 = pairsum(e10) = [se0,se1,po0,po1,q]
    lg5    = ln(ps5): a3 = lg5[:,0:2], log_py = lg5[:,2:4] - lg5[:,4:5]
    alpha_s' = alpha_s - c * (alpha_s - a3_s)    (fused scalar_tensor_tensor + sub)

The predicted-output normalizer q = sum_o po_o collapses to e^{a2_0}+e^{a2_1}
because log_obs is normalized over o; sum_k c = 1 because A rows are a softmax.
"""

import os
from contextlib import ExitStack

import numpy as np

N_PROBLEMS = 10000
N_KCS = 100
BATCH = 1024
T_FULL = 500
N_CORES = 8
BL = BATCH // N_CORES  # 128 students per core

_CHUNK = 8  # time steps per gather slab (8*128 = 1024 = SWDGE ring capacity)


def _log_softmax(x, axis):
    x = x.astype(np.float32)
    m = x.max(axis=axis, keepdims=True)
    e = np.exp(x - m)
    return (x - m) - np.log(e.sum(axis=axis, keepdims=True))


def _wrap_idx(flat):
    """dma_gather index layout: flat index i lives at partition i%16, col i//16,
    replicated across the 8 gpsimd cores (16-partition groups)."""
    assert flat.size % 16 == 0
    w = flat.astype(np.int16).reshape(-1, 16).T  # [16, N/16]
    return np.tile(w, (8, 1))  # [128, N/16]


def _host_tables(A, trans_logits, obs_logits_problem, init_logits):
    P = A.shape[0]
    K = trans_logits.shape[0]
    log_t = _log_softmax(trans_logits, axis=1)  # [K,2,2] normalized over middle axis
    G = A.astype(np.float32) @ log_t.reshape(K, 4)  # [P,4] laid out (s,s')
    L = _log_softmax(obs_logits_problem, axis=2)  # [P,2,2] normalized over outputs

    taba = np.zeros((P, 128), np.float32)
    taba[:, 0:100] = A.astype(np.float32)
    taba[:, 100:104] = G
    # cols 104:106 stay zero (used as the q-slot zeros via tabp instead)

    # tabp row (2p + corr): [M4-scratch | L4 in (o,s) order | zeros | OLL2]
    # cols 0:4 are overwritten on-chip with M4 = G4 + OLL(s'), making
    # cols 0:10 = [M4, L4, Z2] the contiguous input of the t010 add.
    tabp = np.zeros((2 * P, 64), np.float32)
    L4 = np.stack([L[:, 0, 0], L[:, 1, 0], L[:, 0, 1], L[:, 1, 1]], axis=1)
    tabp[0::2, 4:8] = L4
    tabp[1::2, 4:8] = L4
    tabp[0::2, 10:12] = L[:, :, 0]
    tabp[1::2, 10:12] = L[:, :, 1]

    la0 = _log_softmax(init_logits, axis=1)  # [K,2]
    alpha0 = np.empty((BL, 2 * K), np.float32)
    alpha0[:, 0:K] = la0[:, 0]  # s=0 block
    alpha0[:, K:] = la0[:, 1]  # s=1 block
    return taba, tabp, alpha0


def _setup_act_tables():
    """Both Exp and Ln live in the 'natural_log_exp_and_others' ACT table
    set, but the default set ordering makes bacc pick a different set for
    each, inserting a ~2.7us ACT_TABLE_LOAD per activation (2 per time
    step!). Reorder the set list so that set comes first for both bacc's
    chooser and walrus (via BASS_ACT_ROOT_JSON_PATH), collapsing the loads
    to one for the whole kernel."""
    import glob
    import json
    import tempfile

    if os.environ.get("_BKT_ACT_TABLES"):
        return
    from neuronxcc.driver.Job import Job  # pyright: ignore[reportMissingImports]
    from neuronxcc.driver.jobs.support.FindActInfo import (  # pyright: ignore[reportMissingImports]
        findActInfoFile,
    )

    src = findActInfoFile(Job.getPackageDir(), "gen3")
    d = json.load(open(src))
    d["act_func_sets"] = sorted(
        d["act_func_sets"],
        key=lambda s: s["name"] != "natural_log_exp_and_others")
    tmp = tempfile.mkdtemp(prefix="bkt_act_")
    with open(tmp + "/act_info.json", "w") as f:
        json.dump(d, f)
    for p in glob.glob(os.path.dirname(src) + "/*"):
        b = os.path.basename(p)
        if b != "act_info.json":
            os.symlink(p, tmp + "/" + b)
    os.environ["BASS_ACT_ROOT_JSON_PATH"] = tmp + "/act_info.json"
    os.environ["_BKT_ACT_TABLES"] = "1"

    import concourse.bacc as bacc_mod
    import concourse.mybir as mybir

    def tables(arch):
        return {
            e["name"]: {mybir.ActivationFunctionType.from_pwp(v)
                        for v in e["act"].keys()}
            for e in d["act_func_sets"]
        }

    bacc_mod.get_activation_tables = tables


def _emit_program(T, Tc):
    import concourse.mybir as mybir
    import concourse.tile as tile
    from concourse import bacc

    _setup_act_tables()

    f32 = mybir.dt.float32
    i16 = mybir.dt.int16
    Alu = mybir.AluOpType
    Act = mybir.ActivationFunctionType
    K = N_KCS

    nc = bacc.Bacc("TRN2", target_bir_lowering=False, debug=False)

    taba = nc.dram_tensor("taba", [N_PROBLEMS, 128], f32, kind="ExternalInput")
    tabp = nc.dram_tensor("tabp", [2 * N_PROBLEMS, 64], f32, kind="ExternalInput")
    kcw = nc.dram_tensor("kcw", [128, T * 8], i16, kind="ExternalInput")
    ppw = nc.dram_tensor("ppw", [128, T * 8], i16, kind="ExternalInput")
    alpha0 = nc.dram_tensor("alpha0", [BL, 2 * K], f32, kind="ExternalInput")
    out = nc.dram_tensor("out", [BL, T * 2], f32, kind="ExternalOutput")

    # SWDGE descriptor ring fits 1024 descriptors; each gathered row is one
    # descriptor, so cap each dma_gather call at 1024 indices (8 steps).
    assert Tc * 128 <= 1024
    chunks = []  # (t0, tcn)
    t0 = 0
    while t0 < T:
        chunks.append((t0, min(Tc, T - t0)))
        t0 += Tc
    n_chunks = len(chunks)

    from concourse import library_config

    with ExitStack() as ctx:
        tc = ctx.enter_context(tile.TileContext(nc))
        nc.gpsimd.load_library(library_config.mlp)
        idx_pool = ctx.enter_context(tc.tile_pool(name="idx", bufs=1))
        slab_pool = ctx.enter_context(tc.tile_pool(name="slabs", bufs=3))
        state_pool = ctx.enter_context(tc.tile_pool(name="state", bufs=2))
        small_pool = ctx.enter_context(tc.tile_pool(name="small", bufs=4))
        u_pool = ctx.enter_context(tc.tile_pool(name="u", bufs=2))
        out_pool = ctx.enter_context(tc.tile_pool(name="outb", bufs=1))

        kcw_t = idx_pool.tile([128, T * 8], i16, tag="kcw")
        nc.sync.dma_start(kcw_t[:], kcw.ap())
        ppw_t = idx_pool.tile([128, T * 8], i16, tag="ppw")
        nc.sync.dma_start(ppw_t[:], ppw.ap())

        alpha = state_pool.tile([128, 2 * K], f32, tag="alpha")
        nc.sync.dma_start(alpha[:], alpha0.ap())

        outbuf = out_pool.tile([128, T * 2], f32)
        # per-step ln() results land here: [se0, se1, po0, po1, q] per t
        lgbuf = out_pool.tile([128, T * 5], f32)

        slabsA = [None] * n_chunks
        slabsP = [None] * n_chunks
        ni_regs = {}  # distinct chunk sizes get one register each
        for tcn in sorted({c[1] for c in chunks}):
            r = nc.gpsimd.alloc_register(f"ni{tcn}")
            nc.gpsimd.reg_mov(r, tcn * 128)
            ni_regs[tcn] = r

        def issue_gather(n):
            t0, tcn = chunks[n]
            ni = ni_regs[tcn]
            sa = slab_pool.tile([128, Tc, 128], f32, tag="slabA")
            nc.gpsimd.dma_gather(
                sa[:, 0:tcn, :], taba.ap(), kcw_t[:, t0 * 8:(t0 + tcn) * 8],
                num_idxs=tcn * 128, num_idxs_reg=ni, elem_size=128,
            )
            sp = slab_pool.tile([128, Tc, 64], f32, tag="slabP")
            nc.gpsimd.dma_gather(
                sp[:, 0:tcn, :], tabp.ap(), ppw_t[:, t0 * 8:(t0 + tcn) * 8],
                num_idxs=tcn * 128, num_idxs_reg=ni, elem_size=64,
            )
            slabsA[n], slabsP[n] = sa, sp

        issue_gather(0)
        if n_chunks > 1:
            issue_gather(1)

        def c_ap(t):
            return slabsA[t // Tc][:, t % Tc, 0:K]

        def emit_a2(t, alpha_t):
            """a2_s = sum_k c * alpha_s, fused via scalar_tensor_tensor accum.
            alpha is interleaved [128, (k, s)]; stride-2 views pick each s."""
            a2 = small_pool.tile([128, 2], f32, tag="a2")
            for s in range(2):
                u = u_pool.tile([128, K], f32, tag="u")
                nc.vector.scalar_tensor_tensor(
                    out=u[:], in0=c_ap(t), scalar=0.0,
                    in1=alpha_t[:, s * K:(s + 1) * K],
                    op0=Alu.bypass, op1=Alu.mult,
                    accum_out=a2[:, s:s + 1],
                )
            return a2

        # prologue: a2 for t=0
        a2 = emit_a2(0, alpha)

        for n in range(n_chunks):
            if n + 2 < n_chunks:
                issue_gather(n + 2)
            for j in range(chunks[n][1]):
                t = chunks[n][0] + j
                sa, sp = slabsA[n], slabsP[n]
                # M4 = G4 + OLL(s'), written into the slabP row scratch so
                # cols 0:10 become the contiguous [M4, L4, Z2] block
                nc.vector.tensor_tensor(
                    out=sp[:, j, 0:4].rearrange("p (a b) -> p a b", b=2),
                    in0=sa[:, j, 100:104].rearrange("p (a b) -> p a b", b=2),
                    in1=sp[:, j, 10:12].unsqueeze(1).broadcast_to([128, 2, 2]),
                    op=Alu.add,
                )
                # t010 = [M4, L4, Z2] + a2 dup
                t010 = small_pool.tile([128, 10], f32, tag="t010")
                nc.vector.tensor_tensor(
                    out=t010[:].rearrange("p (a b) -> p a b", b=2),
                    in0=sp[:, j, 0:10].rearrange("p (a b) -> p a b", b=2),
                    in1=a2[:].unsqueeze(1).broadcast_to([128, 5, 2]),
                    op=Alu.add,
                )
                e10 = small_pool.tile([128, 10], f32, tag="e10")
                nc.scalar.activation(e10[:], t010[:], Act.Exp)
                ps5 = small_pool.tile([128, 5], f32, tag="ps5")
                ev = e10[:].rearrange("p (a b) -> p a b", b=2)
                nc.vector.tensor_tensor(
                    out=ps5[:], in0=ev[:, :, 0], in1=ev[:, :, 1], op=Alu.add,
                )
                lg5 = lgbuf[:, 5 * t:5 * t + 5]
                nc.scalar.activation(lg5, ps5[:], Act.Ln)
                # state update per s: alpha_s' = alpha_s - c*(alpha_s - a3_s)
                alpha_new = state_pool.tile([128, 2 * K], f32, tag="alpha")
                for s in range(2):
                    r = u_pool.tile([128, K], f32, tag="r")
                    nc.vector.scalar_tensor_tensor(
                        out=r[:], in0=alpha[:, s * K:(s + 1) * K],
                        scalar=lgbuf[:, 5 * t + s:5 * t + s + 1], in1=c_ap(t),
                        op0=Alu.subtract, op1=Alu.mult,
                    )
                    nc.vector.tensor_tensor(
                        out=alpha_new[:, s * K:(s + 1) * K],
                        in0=alpha[:, s * K:(s + 1) * K], in1=r[:],
                        op=Alu.subtract,
                    )
                alpha = alpha_new
                # a2 for next step
                if t + 1 < T:
                    a2 = emit_a2(t + 1, alpha)

        # normalize all outputs at once: log_py[t, o] = lpo[t, o] - lq[t]
        lg3 = lgbuf[:].rearrange("p (t f) -> p t f", f=5)
        nc.vector.tensor_tensor(
            out=outbuf[:].rearrange("p (t o) -> p t o", o=2),
            in0=lg3[:, :, 2:4],
            in1=lg3[:, :, 4:5].broadcast_to([128, T, 2]),
            op=Alu.subtract,
        )
        nc.sync.dma_start(out.ap(), outbuf[:])

    nc.compile()
    return nc


def _prep_inputs(corr, kc, problem, A, trans_logits, obs_logits_problem, init_logits, T):
    corr = np.asarray(corr).astype(np.int64)
    kc = np.asarray(kc).astype(np.int64)
    problem = np.asarray(problem).astype(np.int64)
    taba, tabp, alpha0 = _host_tables(
        np.asarray(A), np.asarray(trans_logits),
        np.asarray(obs_logits_problem), np.asarray(init_logits))

    in_maps = []
    for i in range(N_CORES):
        sl = slice(i * BL, (i + 1) * BL)
        kc_l = kc[sl, :T]  # [128, T]
        pp_l = 2 * problem[sl, :T] + corr[sl, :T]
        # gather flat order: i = j*128 + p  ->  idx = kc_l[p, j]
        kcw = _wrap_idx(kc_l.T.ravel())
        ppw = _wrap_idx(pp_l.T.ravel())
        in_maps.append({
            "taba": taba, "tabp": tabp, "kcw": kcw, "ppw": ppw,
            "alpha0": alpha0,
        })
    return in_maps


def kernel(corr, kc, problem, A, trans_logits, obs_logits_problem, init_logits,
           _T=None, _trace=False):
    T = _T or T_FULL
    nc = _emit_program(T, min(_CHUNK, T))
    in_maps = _prep_inputs(corr, kc, problem, A, trans_logits,
                           obs_logits_problem, init_logits, T)

    from concourse.bass_utils import run_bass_kernel_spmd
    res = run_bass_kernel_spmd(nc, in_maps, core_ids=list(range(N_CORES)),
                               trace=_trace)
    outs = [r["out"].reshape(BL, T, 2) for r in res.results]
    full = np.concatenate(outs, axis=0).astype(np.float32)
    kernel.last_results = res
    return full


if __name__ == "__main__":
    # smoke test on tiny T via CoreSim
    pass

